# revision 16
# baseline (speedup 1.0000x reference)
"""AttentionSharingUnit kernel for 8 Trainium2 cores (Bass/Tile).

Sharding: core = f*4 + b*2 + dh  (frame, batch, d-half). Each core owns 1024
rows of one (frame, batch) and keeps them for the whole kernel.

Phase 1 (spatial attn): QKV projections (+LoRA) in bf16 on TensorE, one
2-rank AllGather of (kT, v) across d-half pairs, per-head attention with
"lazy softmax" (no max subtraction; normalizer = PV matmul against a ones
column appended to V), O-projection (+LoRA) and fp32 residual.

Phase 2 (temporal attn over 2 frames): LayerNorm (gamma/beta folded into Wi
on host), Wi/Wtq/Wtk/Wtv projections, one 2-rank AllGather of (kt, vt)
across frame pairs, 2-way softmax as a sigmoid on VectorE, Wto projection,
final residual.

All matmul inputs bf16, fp32 PSUM accumulate, fp32 residuals.
"""

import os
import sys
from contextlib import ExitStack

import numpy as np

sys.path.insert(0, "/opt/trn_rl_repo")

import ml_dtypes

import concourse.bass as bass
import concourse.tile as tile
from concourse import bacc, mybir
from concourse.bass_utils import run_bass_kernel_spmd
from concourse.masks import make_identity

BF16 = mybir.dt.bfloat16
F32 = mybir.dt.float32
NPBF = ml_dtypes.bfloat16

FRAMES = 2
HEADS = 20
C = 1280
RANK = 256
B = 2
D_FULL = 2048
D = 1024          # rows per core
P = 128
CT = C // P       # 10 c-chunks
DT = D // P       # 8 d-tiles per core
RT = RANK // P    # 2 r-chunks
DH = 64           # head dim
NJ = 16           # j-chunks of 128 over full 2048 keys
EPS = 1e-6
SCALE = DH ** -0.5

EB = [(0, 512), (512, 512), (1024, 256)]   # e-blocks covering 1280
BI_BO, BI_BIP, BI_BTQ, BI_BTK, BI_BTV, BI_BTO = range(6)

RG_DHALF = [[0, 1], [2, 3], [4, 5], [6, 7]]   # phase-1 allgather groups
RG_FRAME = [[0, 4], [1, 5], [2, 6], [3, 7]]   # phase-2 allgather groups

HALF_ELEMS = CT * P * D  # 1310720 elements per packed AG slot


def _build_program():
    nc = bacc.Bacc("TRN2", target_bir_lowering=False, debug=False, num_devices=8)

    def din(name, shape, dt=BF16):
        return nc.dram_tensor(name, list(shape), dt, kind="ExternalInput").ap()

    io = {}
    io["xT"] = din("xT", (C, D))
    io["x32"] = din("x32", (D, C), F32)
    for w in ("wqT", "wkT", "wvT", "woT", "wiT", "wtqT", "wtkT", "wtvT", "wtoT"):
        io[w] = din(w, (C, C))
    for w in ("dqT", "dkT", "dvT", "doT"):
        io[w] = din(w, (C, RANK))
    for w in ("uqT", "ukT", "uvT", "uoT"):
        io[w] = din(w, (RANK, C))
    io["biases"] = din("biases", (1, 6, C))
    out_dram = nc.dram_tensor("out", [D, C], F32, kind="ExternalOutput").ap()

    with tile.TileContext(nc) as tc:
        _emit(tc, nc, io, out_dram)
    nc.compile()
    return nc


def _emit(tc, nc, io, out_dram):
    with ExitStack() as top:
        const_pool = top.enter_context(tc.tile_pool(name="const", bufs=1))
        identity = const_pool.tile([P, P], BF16, name="identity")
        make_identity(nc, identity[:])
        ones_row = const_pool.tile([1, 512], BF16, name="ones_row")
        nc.vector.memset(ones_row[:], 1.0)
        zb = const_pool.tile([P, 1], F32, name="zb")
        nc.vector.memset(zb[:], 0.0)
        epsb = const_pool.tile([P, 1], F32, name="epsb")
        nc.vector.memset(epsb[:], EPS)
        bias_sb = const_pool.tile([1, 6, C], BF16, name="bias_sb")
        nc.sync.dma_start(bias_sb[:], io["biases"][:])

        # PSUM pools (8 banks total)
        pp = top.enter_context(tc.tile_pool(name="pp", bufs=4, space="PSUM"))
        po = top.enter_context(tc.tile_pool(name="po", bufs=2, space="PSUM"))
        pt = top.enter_context(tc.tile_pool(name="pt", bufs=2, space="PSUM"))

        dram = top.enter_context(tc.tile_pool(name="dram", bufs=1, space="DRAM"))
        ag1_in = dram.tile([2, HALF_ELEMS], BF16, name="ag1_in")
        ag1_out = dram.tile([2, 2, HALF_ELEMS], BF16, name="ag1_out")
        ag2_in = dram.tile([2, HALF_ELEMS], BF16, name="ag2_in")
        ag2_out = dram.tile([2, 2, HALF_ELEMS], BF16, name="ag2_out")

        # ---------- generic projection emitters ----------
        def proj_ed(out_sb, nt, x_sb, w_sb, lora=None, bias_idx=None):
            # out[e|r, d] = W.T @ xT : out_sb [P, nt, D]; x_sb [P, CT, D];
            # w_sb [P, CT, nt*P]
            nk = CT + (RT if lora else 0) + (1 if bias_idx is not None else 0)
            for et in range(nt):
                for db in range(2):
                    ps = pp.tile([P, 512], F32, tag="big", name=f"ps{et}_{db}")
                    k = 0
                    for ct in range(CT):
                        k += 1
                        nc.tensor.matmul(
                            ps[:, :],
                            w_sb[:, ct, et * P:(et + 1) * P],
                            x_sb[:, ct, db * 512:(db + 1) * 512],
                            start=(k == 1), stop=(k == nk),
                        )
                    if lora is not None:
                        tT_sb, u_sb = lora
                        for rt in range(RT):
                            k += 1
                            nc.tensor.matmul(
                                ps[:, :],
                                u_sb[:, rt, et * P:(et + 1) * P],
                                tT_sb[:, rt, db * 512:(db + 1) * 512],
                                start=(k == 1), stop=(k == nk),
                            )
                    if bias_idx is not None:
                        k += 1
                        nc.tensor.matmul(
                            ps[:, :],
                            bias_sb[0:1, bias_idx, et * P:(et + 1) * P],
                            ones_row[0:1, 0:512],
                            start=(k == 1), stop=(k == nk),
                        )
                    nc.scalar.copy(out_sb[:, et, db * 512:(db + 1) * 512], ps[:, :])

        def proj_de(x_sb, w_sb, lora=None, bias_idx=None, evict=None):
            # out[d, e] = xT.T @ W : x_sb [P, CT, D]; w_sb [P, CT, C]
            nk = CT + (RT if lora else 0) + (1 if bias_idx is not None else 0)
            for dt in range(DT):
                for (eo, ew) in EB:
                    ps = pp.tile([P, 512], F32, tag="big", name=f"pd{dt}_{eo}")
                    k = 0
                    for ct in range(CT):
                        k += 1
                        nc.tensor.matmul(
                            ps[:, :ew],
                            x_sb[:, ct, dt * P:(dt + 1) * P],
                            w_sb[:, ct, eo:eo + ew],
                            start=(k == 1), stop=(k == nk),
                        )
                    if lora is not None:
                        tT_sb, u_sb = lora
                        for rt in range(RT):
                            k += 1
                            nc.tensor.matmul(
                                ps[:, :ew],
                                tT_sb[:, rt, dt * P:(dt + 1) * P],
                                u_sb[:, rt, eo:eo + ew],
                                start=(k == 1), stop=(k == nk),
                            )
                    if bias_idx is not None:
                        k += 1
                        nc.tensor.matmul(
                            ps[:, :ew],
                            ones_row[0:1, 0:P],
                            bias_sb[0:1, bias_idx, eo:eo + ew],
                            start=(k == 1), stop=(k == nk),
                        )
                    evict(ps, dt, eo, ew)

        def transpose_into(dst_sb, src_sb, nt_src, nt_dst):
            # src [P, nt_src, nt_dst*P] -> dst [P, nt_dst, nt_src*P]
            for st in range(nt_src):
                for ot in range(nt_dst):
                    tp = pt.tile([P, P], BF16, tag="tp", name=f"tp{st}_{ot}")
                    nc.tensor.transpose(
                        tp[:, :], src_sb[:, st, ot * P:(ot + 1) * P], identity[:]
                    )
                    nc.any.tensor_copy(dst_sb[:, ot, st * P:(st + 1) * P], tp[:, :])

        def load_w(pool, name, tag="wfull"):
            t = pool.tile([P, CT, C], BF16, tag=tag, name=f"w_{name}")
            nc.sync.dma_start(t[:], io[name].rearrange("(t p) e -> p t e", p=P))
            return t

        def load_lora(pool, dname, uname):
            dT = pool.tile([P, CT, RANK], BF16, tag="wd", name=f"w_{dname}")
            nc.sync.dma_start(dT[:], io[dname].rearrange("(t p) r -> p t r", p=P))
            uT = pool.tile([P, RT, C], BF16, tag="wu", name=f"w_{uname}")
            nc.sync.dma_start(uT[:], io[uname].rearrange("(t p) e -> p t e", p=P))
            return dT, uT

        # ================= PHASE 1 =================
        p1s = ExitStack()
        ph1 = p1s.enter_context(tc.tile_pool(name="ph1", bufs=1))
        qT_sb = ph1.tile([P, CT, D], BF16, name="qT_sb")
        o_normT = ph1.tile([P, CT, D], BF16, name="o_normT")

        with ExitStack() as s1:
            wpool = s1.enter_context(tc.tile_pool(name="wpool1", bufs=1))
            lpool = s1.enter_context(tc.tile_pool(name="lpool1", bufs=2))
            tpool = s1.enter_context(tc.tile_pool(name="tpool1", bufs=2))
            kvpool = s1.enter_context(tc.tile_pool(name="kvpool1", bufs=1))
            xpool = s1.enter_context(tc.tile_pool(name="xpool1", bufs=1))

            xT_sb = xpool.tile([P, CT, D], BF16, name="xT_sb")
            nc.sync.dma_start(xT_sb[:], io["xT"].rearrange("(t p) d -> p t d", p=P))

            # ---- k projection (own half) ----
            w = load_w(wpool, "wkT")
            dTw, uTw = load_lora(lpool, "dkT", "ukT")
            tkT = tpool.tile([P, RT, D], BF16, tag="tT", name="tkT")
            proj_ed(tkT, RT, xT_sb, dTw)
            kT_own = kvpool.tile([P, CT, D], BF16, tag="kvown", name="kT_own")
            proj_ed(kT_own, CT, xT_sb, w, lora=(tkT, uTw))
            nc.sync.dma_start(
                ag1_in[0].rearrange("(t p d) -> p t d", p=P, d=D), kT_own[:]
            )

            # ---- v projection (own half) ----
            w = load_w(wpool, "wvT")
            dTw, uTw = load_lora(lpool, "dvT", "uvT")
            tvT = tpool.tile([P, RT, D], BF16, tag="tT", name="tvT")
            proj_ed(tvT, RT, xT_sb, dTw)
            v_own = kvpool.tile([P, DT, C], BF16, tag="kvown", name="v_own")

            def v_evict(ps, dt, eo, ew):
                nc.scalar.copy(v_own[:, dt, eo:eo + ew], ps[:, :ew])

            proj_de(xT_sb, w, lora=(tvT, uTw), evict=v_evict)
            nc.sync.dma_start(
                ag1_in[1].rearrange("(t p e) -> p t e", p=P, e=C), v_own[:]
            )

            # ---- allgather (kT, v) across the d-half pair ----
            nc.gpsimd.collective_compute(
                "AllGather",
                mybir.AluOpType.bypass,
                replica_groups=RG_DHALF,
                ins=[ag1_in[:].opt()],
                outs=[ag1_out[:].opt()],
            )

            # ---- q projection (own rows; overlaps the collective) ----
            w = load_w(wpool, "wqT")
            dTw, uTw = load_lora(lpool, "dqT", "uqT")
            tqT = tpool.tile([P, RT, D], BF16, tag="tT", name="tqT")
            proj_ed(tqT, RT, xT_sb, dTw)
            proj_ed(qT_sb, CT, xT_sb, w, lora=(tqT, uTw))

        # ---- attention: stream per-head-pair k/v from the AG dram buffer ----
        with ExitStack() as s2:
            khp = s2.enter_context(tc.tile_pool(name="khp", bufs=2))
            vhp = s2.enter_context(tc.tile_pool(name="vhp", bufs=3))
            epool = s2.enter_context(tc.tile_pool(name="epool", bufs=2))
            spool = s2.enter_context(tc.tile_pool(name="spool", bufs=4))
            opp = s2.enter_context(tc.tile_pool(name="opp", bufs=2))
            for hc in range(CT):  # head-pair = one 128-channel chunk
                kh2 = khp.tile([P, D_FULL], BF16, tag="kh", name=f"kh{hc}")
                for g in range(2):
                    nc.sync.dma_start(
                        kh2[:, g * D:(g + 1) * D],
                        ag1_out[g, 0].rearrange("(t p d) -> p t d", p=P, d=D)[:, hc],
                    )
                o_pair = opp.tile([P, DT, P], BF16, tag="opair", name=f"op{hc}")
                for h01 in range(2):
                    hh = 2 * hc + h01
                    poff = h01 * DH
                    vh = vhp.tile([P, NJ, 65], BF16, tag="vh", name=f"vh{hh}")
                    for g in range(2):
                        nc.sync.dma_start(
                            vh[:, g * DT:(g + 1) * DT, 0:DH],
                            ag1_out[g, 1].rearrange("(t p e) -> p t e", p=P, e=C)[
                                :, :, hh * DH:(hh + 1) * DH
                            ],
                        )
                    nc.vector.memset(vh[:, :, DH:65], 1.0)
                    for ib in range(2):
                        exp_t = epool.tile(
                            [P, NJ, 512], BF16, tag="exp", name=f"e{hh}_{ib}"
                        )
                        for jc in range(NJ):
                            sim = pp.tile(
                                [P, 512], F32, tag="big", name=f"s{hh}{ib}{jc}"
                            )
                            nc.tensor.matmul(
                                sim[:, :],
                                kh2[poff:poff + DH, jc * P:(jc + 1) * P],
                                qT_sb[poff:poff + DH, hc,
                                      ib * 512:(ib + 1) * 512],
                                start=True, stop=True,
                            )
                            nc.scalar.activation(
                                exp_t[:, jc, :], sim[:, :],
                                mybir.ActivationFunctionType.Exp,
                                bias=zb[:], scale=SCALE,
                            )
                        for it in range(4):
                            dt = ib * 4 + it
                            ops = po.tile(
                                [P, 65], F32, tag="opsum", name=f"o{hh}_{dt}"
                            )
                            for jc in range(NJ):
                                nc.tensor.matmul(
                                    ops[:, :],
                                    exp_t[:, jc, it * P:(it + 1) * P],
                                    vh[:, jc, :],
                                    start=(jc == 0), stop=(jc == NJ - 1),
                                )
                            rec = spool.tile(
                                [P, 1], F32, tag="rec", name=f"r{hh}_{dt}"
                            )
                            nc.vector.reciprocal(rec[:], ops[:, DH:65])
                            nc.vector.tensor_scalar_mul(
                                o_pair[:, dt, poff:poff + DH], ops[:, 0:DH], rec[:]
                            )
                # transpose the pair's [d, 128c] block into o_normT [c, d]
                for dt in range(DT):
                    tp = pt.tile([P, P], BF16, tag="tp", name=f"t{hc}_{dt}")
                    nc.tensor.transpose(tp[:, :], o_pair[:, dt, :], identity[:])
                    nc.any.tensor_copy(
                        o_normT[:, hc, dt * P:(dt + 1) * P], tp[:, :]
                    )

        # ---- O projection + residual -> mh ----
        ph2 = top.enter_context(tc.tile_pool(name="ph2", bufs=1, side="right"))
        mh_sb = ph2.tile([P, DT, C], F32, name="mh_sb")

        with ExitStack() as s3:
            wpool = s3.enter_context(tc.tile_pool(name="wpool3", bufs=1))
            lpool = s3.enter_context(tc.tile_pool(name="lpool3", bufs=1))
            tpool = s3.enter_context(tc.tile_pool(name="tpool3", bufs=1))
            x32p = s3.enter_context(tc.tile_pool(name="x32p", bufs=3))

            w = load_w(wpool, "woT")
            dTw, uTw = load_lora(lpool, "doT", "uoT")
            toT = tpool.tile([P, RT, D], BF16, tag="toT", name="toT")
            proj_ed(toT, RT, o_normT, dTw)

            def o_evict(ps, dt, eo, ew):
                x32t = x32p.tile([P, 512], F32, tag="x32t", name=f"xo{dt}_{eo}")
                nc.sync.dma_start(
                    x32t[:, :ew],
                    io["x32"].rearrange("(t p) e -> p t e", p=P)[:, dt, eo:eo + ew],
                )
                nc.vector.tensor_add(
                    mh_sb[:, dt, eo:eo + ew], ps[:, :ew], x32t[:, :ew]
                )

            proj_de(o_normT, w, lora=(toT, uTw), bias_idx=BI_BO, evict=o_evict)
        p1s.close()  # free qT_sb / o_normT

        # ================= PHASE 2 =================
        qt_sb = ph2.tile([P, DT, C], BF16, name="qt_sb")
        xo_sb = ph2.tile([P, DT, C], BF16, name="xo_sb")

        with ExitStack() as s4:
            wpool = s4.enter_context(tc.tile_pool(name="wpool4", bufs=1))
            xtp = s4.enter_context(tc.tile_pool(name="xtp", bufs=1))
            xiT = xtp.tile([P, CT, D], BF16, tag="xiT", name="xiT")

            with ExitStack() as s4a:
                lnp = s4a.enter_context(tc.tile_pool(name="lnp", bufs=2))
                xnp = s4a.enter_context(tc.tile_pool(name="xnp", bufs=1))
                xnT = xnp.tile([P, CT, D], BF16, name="xnT")
                for dt in range(DT):
                    row = mh_sb[:, dt, :]
                    ssum = lnp.tile([P, 1], F32, tag="s1", name=f"su{dt}")
                    nc.vector.reduce_sum(ssum[:], row, axis=mybir.AxisListType.X)
                    mu = lnp.tile([P, 1], F32, tag="s2", name=f"mu{dt}")
                    nc.vector.tensor_scalar_mul(mu[:], ssum[:], 1.0 / C)
                    xm = lnp.tile([P, C], F32, tag="xm", name=f"xm{dt}")
                    nc.vector.tensor_scalar(
                        xm[:], row, mu[:], None, mybir.AluOpType.subtract
                    )
                    sq = lnp.tile([P, C], BF16, tag="sq", name=f"sq{dt}")
                    ssq = lnp.tile([P, 1], F32, tag="s3", name=f"sv{dt}")
                    nc.scalar.activation(
                        sq[:], xm[:], mybir.ActivationFunctionType.Square,
                        bias=zb[:], accum_out=ssq[:],
                    )
                    stdt = lnp.tile([P, 1], F32, tag="s4", name=f"sd{dt}")
                    nc.scalar.activation(
                        stdt[:], ssq[:], mybir.ActivationFunctionType.Sqrt,
                        scale=1.0 / C, bias=epsb[:],
                    )
                    rstd = lnp.tile([P, 1], F32, tag="s5", name=f"rs{dt}")
                    nc.vector.reciprocal(rstd[:], stdt[:])
                    xn_t = lnp.tile([P, C], BF16, tag="xn", name=f"xn{dt}")
                    nc.vector.tensor_scalar_mul(xn_t[:], xm[:], rstd[:])
                    # transpose this row-tile straight into xnT
                    for ct in range(CT):
                        tp = pt.tile([P, P], BF16, tag="tp", name=f"tn{dt}_{ct}")
                        nc.tensor.transpose(
                            tp[:, :], xn_t[:, ct * P:(ct + 1) * P], identity[:]
                        )
                        nc.any.tensor_copy(
                            xnT[:, ct, dt * P:(dt + 1) * P], tp[:, :]
                        )
                w = load_w(wpool, "wiT")
                proj_ed(xiT, CT, xnT, w, bias_idx=BI_BIP)

            # ---- temporal k/v/q projections ----
            with ExitStack() as s4b:
                kvtp = s4b.enter_context(tc.tile_pool(name="kvtp", bufs=1))
                kt_sb = kvtp.tile([P, DT, C], BF16, tag="kt", name="kt_sb")
                vt_sb = kvtp.tile([P, DT, C], BF16, tag="vt", name="vt_sb")

                w = load_w(wpool, "wtkT")

                def kt_evict(ps, dt, eo, ew):
                    nc.scalar.copy(kt_sb[:, dt, eo:eo + ew], ps[:, :ew])

                proj_de(xiT, w, bias_idx=BI_BTK, evict=kt_evict)
                nc.sync.dma_start(
                    ag2_in[0].rearrange("(t p e) -> p t e", p=P, e=C), kt_sb[:]
                )

                w = load_w(wpool, "wtvT")

                def vt_evict(ps, dt, eo, ew):
                    nc.scalar.copy(vt_sb[:, dt, eo:eo + ew], ps[:, :ew])

                proj_de(xiT, w, bias_idx=BI_BTV, evict=vt_evict)
                nc.sync.dma_start(
                    ag2_in[1].rearrange("(t p e) -> p t e", p=P, e=C), vt_sb[:]
                )

                nc.gpsimd.collective_compute(
                    "AllGather",
                    mybir.AluOpType.bypass,
                    replica_groups=RG_FRAME,
                    ins=[ag2_in[:].opt()],
                    outs=[ag2_out[:].opt()],
                )

                w = load_w(wpool, "wtqT")

                def qt_evict(ps, dt, eo, ew):
                    nc.scalar.copy(qt_sb[:, dt, eo:eo + ew], ps[:, :ew])

                proj_de(xiT, w, bias_idx=BI_BTQ, evict=qt_evict)

        # ---- temporal attention (seq len 2 -> sigmoid weights) ----
        with ExitStack() as s5:
            kvp = s5.enter_context(tc.tile_pool(name="kvp", bufs=3))
            ap = s5.enter_context(tc.tile_pool(name="ap", bufs=3))
            for dt in range(DT):
                k0 = kvp.tile([P, C], BF16, tag="k0", name=f"k0_{dt}")
                k1 = kvp.tile([P, C], BF16, tag="k1", name=f"k1_{dt}")
                v0 = kvp.tile([P, C], BF16, tag="v0", name=f"v0_{dt}")
                v1 = kvp.tile([P, C], BF16, tag="v1", name=f"v1_{dt}")
                for t, (g, s) in zip((k0, k1, v0, v1),
                                     ((0, 0), (1, 0), (0, 1), (1, 1))):
                    nc.sync.dma_start(
                        t[:],
                        ag2_out[g, s].rearrange("(t p e) -> p t e", p=P, e=C)[:, dt],
                    )
                qrow = qt_sb[:, dt, :]
                prod = ap.tile([P, C], F32, tag="prod", name=f"pr{dt}")
                s0 = ap.tile([P, HEADS], F32, tag="s0", name=f"s0_{dt}")
                s1v = ap.tile([P, HEADS], F32, tag="s1v", name=f"s1_{dt}")
                nc.vector.tensor_mul(prod[:], qrow, k0[:])
                nc.vector.reduce_sum(
                    s0[:], prod.rearrange("p (h w) -> p h w", w=DH),
                    axis=mybir.AxisListType.X,
                )
                nc.vector.tensor_mul(prod[:], qrow, k1[:])
                nc.vector.reduce_sum(
                    s1v[:], prod.rearrange("p (h w) -> p h w", w=DH),
                    axis=mybir.AxisListType.X,
                )
                sd = ap.tile([P, HEADS], F32, tag="sd", name=f"sd{dt}")
                nc.vector.tensor_sub(sd[:], s1v[:], s0[:])
                w1 = ap.tile([P, HEADS], F32, tag="w1", name=f"w1_{dt}")
                nc.scalar.activation(
                    w1[:], sd[:], mybir.ActivationFunctionType.Sigmoid,
                    bias=zb[:], scale=SCALE,
                )
                dv = ap.tile([P, C], F32, tag="dv", name=f"dv{dt}")
                nc.vector.tensor_sub(dv[:], v1[:], v0[:])
                # xo = v0 + w1 * (v1 - v0), per 64-wide head segment
                for hh in range(HEADS):
                    seg = slice(hh * DH, (hh + 1) * DH)
                    nc.vector.tensor_scalar_mul(
                        prod[:, seg], dv[:, seg], w1[:, hh:hh + 1]
                    )
                    nc.vector.tensor_add(xo_sb[:, dt, seg], prod[:, seg], v0[:, seg])

        # ---- final projection + residual ----
        with ExitStack() as s6:
            wpool = s6.enter_context(tc.tile_pool(name="wpool6", bufs=1))
            xtp6 = s6.enter_context(tc.tile_pool(name="xtp6", bufs=1))
            fin = s6.enter_context(tc.tile_pool(name="fin", bufs=3))

            xoT = xtp6.tile([P, CT, D], BF16, name="xoT")
            transpose_into(xoT, xo_sb, DT, CT)
            w = load_w(wpool, "wtoT")

            def fin_evict(ps, dt, eo, ew):
                x32t = fin.tile([P, 512], F32, tag="x32t", name=f"xx{dt}_{eo}")
                nc.sync.dma_start(
                    x32t[:, :ew],
                    io["x32"].rearrange("(t p) e -> p t e", p=P)[:, dt, eo:eo + ew],
                )
                ot = fin.tile([P, 512], F32, tag="ot", name=f"ot{dt}_{eo}")
                nc.vector.tensor_add(ot[:, :ew], ps[:, :ew], mh_sb[:, dt, eo:eo + ew])
                nc.vector.tensor_sub(ot[:, :ew], ot[:, :ew], x32t[:, :ew])
                nc.sync.dma_start(
                    out_dram.rearrange("(t p) e -> p t e", p=P)[:, dt, eo:eo + ew],
                    ot[:, :ew],
                )

            proj_de(xoT, w, bias_idx=BI_BTO, evict=fin_evict)


_NC_CACHE = None


def _get_program():
    global _NC_CACHE
    if _NC_CACHE is None:
        _NC_CACHE = _build_program()
    return _NC_CACHE


def _prep_in_maps(h, Wq, Wk, Wv, Wo, bo, Dq, Uq, Dk, Uk, Dv, Uv, Do, Uo,
                  gamma, beta, Wi, bi, Wtq, btq, Wtk, btk, Wtv, btv, Wto, bto):
    def tb(x):
        return np.ascontiguousarray(np.asarray(x, np.float32).T).astype(NPBF)

    gamma = np.asarray(gamma, np.float32)
    beta = np.asarray(beta, np.float32)
    Wi = np.asarray(Wi, np.float32)
    WiP = Wi * gamma[None, :]
    biP = np.asarray(bi, np.float32) + Wi @ beta

    shared = {
        "wqT": tb(Wq), "wkT": tb(Wk), "wvT": tb(Wv), "woT": tb(Wo),
        "wiT": tb(WiP), "wtqT": tb(Wtq), "wtkT": tb(Wtk), "wtvT": tb(Wtv),
        "wtoT": tb(Wto),
        "biases": np.stack([
            np.asarray(bo, np.float32), biP,
            np.asarray(btq, np.float32), np.asarray(btk, np.float32),
            np.asarray(btv, np.float32), np.asarray(bto, np.float32),
        ])[None].astype(NPBF),
    }
    lora = []
    for f in range(FRAMES):
        lora.append({
            "dqT": tb(Dq[f]), "uqT": tb(Uq[f]),
            "dkT": tb(Dk[f]), "ukT": tb(Uk[f]),
            "dvT": tb(Dv[f]), "uvT": tb(Uv[f]),
            "doT": tb(Do[f]), "uoT": tb(Uo[f]),
        })

    h = np.asarray(h, np.float32)
    in_maps = []
    for core in range(8):
        f, b, dh = core // 4, (core // 2) % 2, core % 2
        x32 = np.ascontiguousarray(h[b * FRAMES + f, dh * D:(dh + 1) * D, :])
        m = dict(shared)
        m.update(lora[f])
        m["x32"] = x32
        m["xT"] = np.ascontiguousarray(x32.T).astype(NPBF)
        in_maps.append(m)
    return in_maps


def kernel(h, **kw):
    nc = _get_program()
    in_maps = _prep_in_maps(h, **kw)
    res = run_bass_kernel_spmd(nc, in_maps, list(range(8))).results
    out = np.empty((B * FRAMES, D_FULL, C), np.float32)
    for core in range(8):
        f, b, dh = core // 4, (core // 2) % 2, core % 2
        out[b * FRAMES + f, dh * D:(dh + 1) * D, :] = res[core]["out"]
    return out


# revision 34
# speedup vs baseline: 95.8026x; 95.8026x over previous
"""AttentionSharingUnit kernel for 8 Trainium2 cores (Bass/Tile).

Sharding: core = f*4 + b*2 + dh  (frame, batch, d-half). Each core owns 1024
rows of one (frame, batch) and keeps them for the whole kernel.

Phase 1 (spatial attn): QKV projections (+LoRA) in bf16 on TensorE, one
2-rank AllGather of (kT, v) across d-half pairs, per-head attention with
"lazy softmax" (no max subtraction; normalizer = PV matmul against a ones
column appended to V), O-projection (+LoRA) and fp32 residual.

Phase 2 (temporal attn over 2 frames): LayerNorm (gamma/beta folded into Wi
on host), Wi/Wtq/Wtk/Wtv projections, one 2-rank AllGather of (kt, vt)
across frame pairs, 2-way softmax as a sigmoid on VectorE, Wto projection,
final residual.

All matmul inputs bf16, fp32 PSUM accumulate, fp32 residuals.
"""

import os
import sys
from contextlib import ExitStack

import numpy as np

sys.path.insert(0, "/opt/trn_rl_repo")

import ml_dtypes

import concourse.bass as bass
import concourse.tile as tile
from concourse import bacc, mybir
from concourse.bass_utils import run_bass_kernel_spmd
from concourse.masks import make_identity

BF16 = mybir.dt.bfloat16
F32 = mybir.dt.float32
NPBF = ml_dtypes.bfloat16

FRAMES = 2
HEADS = 20
C = 1280
RANK = 256
B = 2
D_FULL = 2048
D = 1024          # rows per core
P = 128
CT = C // P       # 10 c-chunks
DT = D // P       # 8 d-tiles per core
RT = RANK // P    # 2 r-chunks
DH = 64           # head dim
NJ = 16           # j-chunks of 128 over full 2048 keys
EPS = 1e-6
SCALE = DH ** -0.5

EB = [(0, 512), (512, 512), (1024, 256)]   # e-blocks covering 1280
BI_BO, BI_BIP, BI_BTQ, BI_BTK, BI_BTV, BI_BTO = range(6)

RG_DHALF = [[0, 1], [2, 3], [4, 5], [6, 7]]   # phase-1 allgather groups
RG_FRAME = [[0, 4], [1, 5], [2, 6], [3, 7]]   # phase-2 allgather groups

HALF_ELEMS = CT * P * D  # 1310720 elements per packed AG slot

# When True, collectives are replaced by local DRAM->DRAM copies with the
# same buffer sizes (for single-core TimelineSim cost-model introspection).
EMULATE_COLLECTIVES = False
# When True, omit the K=1 bias matmuls (exact when all biases are zero;
# decided on the host per input values).
SKIP_BIAS = False


def _build_program(n_iters=1):
    nc = bacc.Bacc("TRN2", target_bir_lowering=False, debug=False, num_devices=8)

    def din(name, shape, dt=BF16):
        return nc.dram_tensor(name, list(shape), dt, kind="ExternalInput").ap()

    io = {}
    io["xT"] = din("xT", (C, D))
    io["x32"] = din("x32", (D, C), F32)
    for w in ("wqT", "wkT", "wvT", "woT", "wtqT", "wtkT", "wtvT", "wtoT"):
        io[w] = din(w, (C, C))
    io["biases"] = din("biases", (1, 6, C))
    out_dram = nc.dram_tensor("out", [D, C], F32, kind="ExternalOutput").ap()

    with tile.TileContext(nc) as tc:
        for _ in range(n_iters):
            _emit(tc, nc, io, out_dram)
    nc.compile()
    return nc


def _bias(i):
    return None if SKIP_BIAS else i


def _emit(tc, nc, io, out_dram):
    with ExitStack() as top:
        const_pool = top.enter_context(tc.tile_pool(name="const", bufs=1))
        identity = const_pool.tile([P, P], BF16, name="identity")
        make_identity(nc, identity[:])
        ones_row = const_pool.tile([1, 512], BF16, name="ones_row")
        nc.vector.memset(ones_row[:], 1.0)
        zb = const_pool.tile([P, 1], F32, name="zb")
        nc.vector.memset(zb[:], 0.0)
        epsb = const_pool.tile([P, 1], F32, name="epsb")
        nc.vector.memset(epsb[:], EPS)
        bias_sb = const_pool.tile([1, 6, C], BF16, name="bias_sb")
        nc.sync.dma_start(bias_sb[:], io["biases"][:])

        # PSUM pools (8 banks total)
        pp = top.enter_context(tc.tile_pool(name="pp", bufs=3, space="PSUM"))
        po = top.enter_context(tc.tile_pool(name="po", bufs=3, space="PSUM"))
        pt = top.enter_context(tc.tile_pool(name="pt", bufs=2, space="PSUM"))

        dram = top.enter_context(tc.tile_pool(name="dram", bufs=1, space="DRAM"))
        ag1k_in = dram.tile([HALF_ELEMS], BF16, name="ag1k_in")
        ag1k_out = dram.tile([2, HALF_ELEMS], BF16, name="ag1k_out")
        ag1v_in = dram.tile([HALF_ELEMS], BF16, name="ag1v_in")
        ag1v_out = dram.tile([2, HALF_ELEMS], BF16, name="ag1v_out")
        ag2k_in = dram.tile([HALF_ELEMS], BF16, name="ag2k_in")
        ag2k_out = dram.tile([2, HALF_ELEMS], BF16, name="ag2k_out")
        ag2v_in = dram.tile([HALF_ELEMS], BF16, name="ag2v_in")
        ag2v_out = dram.tile([2, HALF_ELEMS], BF16, name="ag2v_out")

        def allgather(in_t, out_t, groups):
            if EMULATE_COLLECTIVES:
                nc.sync.dma_start(out_t[0], in_t[:])
                nc.sync.dma_start(out_t[1], in_t[:])
            else:
                nc.gpsimd.collective_compute(
                    "AllGather",
                    mybir.AluOpType.bypass,
                    replica_groups=groups,
                    ins=[in_t[:].opt()],
                    outs=[out_t[:].opt()],
                )

        # ---------- generic projection emitters ----------
        _evict_rr = [0]

        def evict_copy(dst, src_ps):
            _evict_rr[0] ^= 1
            eng = nc.scalar if _evict_rr[0] else nc.vector
            if eng is nc.scalar:
                nc.scalar.copy(dst, src_ps)
            else:
                nc.vector.tensor_copy(dst, src_ps)

        def proj_ed(out_sb, nt, x_sb, w_sb, lora=None, bias_idx=None):
            # out[e|r, d] = W.T @ xT : out_sb [P, nt, D]; x_sb [P, CT, D];
            # w_sb [P, CT, nt*P]
            nk = CT + (RT if lora else 0) + (1 if bias_idx is not None else 0)
            for et in range(nt):
                for db in range(2):
                    ps = pp.tile([P, 512], F32, tag="big", name=f"ps{et}_{db}")
                    k = 0
                    for ct in range(CT):
                        k += 1
                        nc.tensor.matmul(
                            ps[:, :],
                            w_sb[:, ct, et * P:(et + 1) * P],
                            x_sb[:, ct, db * 512:(db + 1) * 512],
                            start=(k == 1), stop=(k == nk),
                        )
                    if lora is not None:
                        tT_sb, u_sb = lora
                        for rt in range(RT):
                            k += 1
                            nc.tensor.matmul(
                                ps[:, :],
                                u_sb[:, rt, et * P:(et + 1) * P],
                                tT_sb[:, rt, db * 512:(db + 1) * 512],
                                start=(k == 1), stop=(k == nk),
                            )
                    if bias_idx is not None:
                        k += 1
                        nc.tensor.matmul(
                            ps[:, :],
                            bias_sb[0:1, bias_idx, et * P:(et + 1) * P],
                            ones_row[0:1, 0:512],
                            start=(k == 1), stop=(k == nk),
                        )
                    evict_copy(out_sb[:, et, db * 512:(db + 1) * 512], ps[:, :])

        def proj_de(x_sb, w_sb, lora=None, bias_idx=None, evict=None):
            # out[d, e] = xT.T @ W : x_sb [P, CT, D]; w_sb [P, CT, C]
            nk = CT + (RT if lora else 0) + (1 if bias_idx is not None else 0)
            for dt in range(DT):
                for (eo, ew) in EB:
                    ps = pp.tile([P, 512], F32, tag="big", name=f"pd{dt}_{eo}")
                    k = 0
                    for ct in range(CT):
                        k += 1
                        nc.tensor.matmul(
                            ps[:, :ew],
                            x_sb[:, ct, dt * P:(dt + 1) * P],
                            w_sb[:, ct, eo:eo + ew],
                            start=(k == 1), stop=(k == nk),
                        )
                    if lora is not None:
                        tT_sb, u_sb = lora
                        for rt in range(RT):
                            k += 1
                            nc.tensor.matmul(
                                ps[:, :ew],
                                tT_sb[:, rt, dt * P:(dt + 1) * P],
                                u_sb[:, rt, eo:eo + ew],
                                start=(k == 1), stop=(k == nk),
                            )
                    if bias_idx is not None:
                        k += 1
                        nc.tensor.matmul(
                            ps[:, :ew],
                            ones_row[0:1, 0:P],
                            bias_sb[0:1, bias_idx, eo:eo + ew],
                            start=(k == 1), stop=(k == nk),
                        )
                    evict(ps, dt, eo, ew)

        def transpose_into(dst_sb, src_sb, nt_src, nt_dst):
            # src [P, nt_src, nt_dst*P] -> dst [P, nt_dst, nt_src*P]
            for st in range(nt_src):
                for ot in range(nt_dst):
                    tp = pt.tile([P, P], BF16, tag="tp", name=f"tp{st}_{ot}")
                    nc.tensor.transpose(
                        tp[:, :], src_sb[:, st, ot * P:(ot + 1) * P], identity[:]
                    )
                    nc.any.tensor_copy(dst_sb[:, ot, st * P:(st + 1) * P], tp[:, :])

        def load_w(pool, name, tag="wfull"):
            t = pool.tile([P, CT, C], BF16, tag=tag, name=f"w_{name}")
            nc.sync.dma_start(t[:], io[name].rearrange("(t p) e -> p t e", p=P))
            return t

        # ================= PHASE 1 =================
        p1s = ExitStack()
        ph1 = p1s.enter_context(tc.tile_pool(name="ph1", bufs=1))
        qT_sb = ph1.tile([P, CT, D], BF16, name="qT_sb")
        o_normT = ph1.tile([P, CT, D], BF16, name="o_normT")

        with ExitStack() as s1:
            wpool = s1.enter_context(tc.tile_pool(name="wpool1", bufs=2))
            kvpool = s1.enter_context(tc.tile_pool(name="kvpool1", bufs=1))
            xpool = s1.enter_context(tc.tile_pool(name="xpool1", bufs=1))

            xT_sb = xpool.tile([P, CT, D], BF16, name="xT_sb")
            nc.sync.dma_start(xT_sb[:], io["xT"].rearrange("(t p) d -> p t d", p=P))

            # ---- k projection (own half) ----
            w = load_w(wpool, "wkT")
            kT_own = kvpool.tile([P, CT, D], BF16, tag="kvown", name="kT_own")
            proj_ed(kT_own, CT, xT_sb, w)
            nc.sync.dma_start(
                ag1k_in.rearrange("(t p d) -> p t d", p=P, d=D), kT_own[:]
            )
            allgather(ag1k_in, ag1k_out, RG_DHALF)

            # ---- v projection (own half) ----
            w = load_w(wpool, "wvT")
            v_own = kvpool.tile([P, DT, C], BF16, tag="kvown", name="v_own")

            def v_evict(ps, dt, eo, ew):
                evict_copy(v_own[:, dt, eo:eo + ew], ps[:, :ew])

            proj_de(xT_sb, w, evict=v_evict)
            nc.sync.dma_start(
                ag1v_in.rearrange("(t p e) -> p t e", p=P, e=C), v_own[:]
            )
            allgather(ag1v_in, ag1v_out, RG_DHALF)

            # ---- q projection (own rows; overlaps the collective) ----
            w = load_w(wpool, "wqT")
            proj_ed(qT_sb, CT, xT_sb, w)

        # ---- attention: stream per-head-pair k/v from the AG dram buffer ----
        with ExitStack() as s2:
            khp = s2.enter_context(tc.tile_pool(name="khp", bufs=2))
            vhp = s2.enter_context(tc.tile_pool(name="vhp", bufs=3))
            epool = s2.enter_context(tc.tile_pool(name="epool", bufs=3))
            spool = s2.enter_context(tc.tile_pool(name="spool", bufs=4))
            opp = s2.enter_context(tc.tile_pool(name="opp", bufs=2))
            for hc in range(CT):  # head-pair = one 128-channel chunk
                kh2 = khp.tile([P, D_FULL], BF16, tag="kh", name=f"kh{hc}")
                for g in range(2):
                    nc.sync.dma_start(
                        kh2[:, g * D:(g + 1) * D],
                        ag1k_out[g].rearrange("(t p d) -> p t d", p=P, d=D)[:, hc],
                    )
                o_pair = opp.tile([P, DT, P], BF16, tag="opair", name=f"op{hc}")
                for h01 in range(2):
                    hh = 2 * hc + h01
                    poff = h01 * DH
                    vh = vhp.tile([P, NJ, 65], BF16, tag="vh", name=f"vh{hh}")
                    for g in range(2):
                        nc.sync.dma_start(
                            vh[:, g * DT:(g + 1) * DT, 0:DH],
                            ag1v_out[g].rearrange("(t p e) -> p t e", p=P, e=C)[
                                :, :, hh * DH:(hh + 1) * DH
                            ],
                        )
                    nc.vector.memset(vh[:, :, DH:65], 1.0)
                    for ib in range(2):
                        exp_t = epool.tile(
                            [P, NJ, 512], BF16, tag="exp", name=f"e{hh}_{ib}"
                        )
                        for jc in range(NJ):
                            sim = pp.tile(
                                [P, 512], F32, tag="big", name=f"s{hh}{ib}{jc}"
                            )
                            nc.tensor.matmul(
                                sim[:, :],
                                kh2[poff:poff + DH, jc * P:(jc + 1) * P],
                                qT_sb[poff:poff + DH, hc,
                                      ib * 512:(ib + 1) * 512],
                                start=True, stop=True,
                            )
                            nc.scalar.activation(
                                exp_t[:, jc, :], sim[:, :],
                                mybir.ActivationFunctionType.Exp,
                                bias=zb[:], scale=SCALE,
                            )
                        # PV with exp stationary (N=65 streams; LDWEIGHTS are
                        # hidden by the engine queue, so this is stream-optimal)
                        for it in range(4):
                            dt = ib * 4 + it
                            ops = po.tile(
                                [P, 65], F32, tag="opsum", name=f"o{hh}_{dt}"
                            )
                            for jc in range(NJ):
                                nc.tensor.matmul(
                                    ops[:, :],
                                    exp_t[:, jc, it * P:(it + 1) * P],
                                    vh[:, jc, :],
                                    start=(jc == 0), stop=(jc == NJ - 1),
                                )
                            rec = spool.tile(
                                [P, 1], F32, tag="rec", name=f"r{hh}_{dt}"
                            )
                            nc.vector.reciprocal(rec[:], ops[:, DH:65])
                            nc.vector.tensor_scalar_mul(
                                o_pair[:, dt, poff:poff + DH], ops[:, 0:DH], rec[:]
                            )
                # transpose the pair's [d, 128c] block into o_normT [c, d]
                for dt in range(DT):
                    tp = pt.tile([P, P], BF16, tag="tp", name=f"t{hc}_{dt}")
                    nc.tensor.transpose(tp[:, :], o_pair[:, dt, :], identity[:])
                    nc.any.tensor_copy(
                        o_normT[:, hc, dt * P:(dt + 1) * P], tp[:, :]
                    )

        # ---- O projection + residual -> mh ----
        ph2 = top.enter_context(tc.tile_pool(name="ph2", bufs=1, side="right"))
        mh_sb = ph2.tile([P, DT, C], F32, name="mh_sb")

        with ExitStack() as s3:
            wpool = s3.enter_context(tc.tile_pool(name="wpool3", bufs=1))
            x32p = s3.enter_context(tc.tile_pool(name="x32p", bufs=3))

            w = load_w(wpool, "woT")

            def o_evict(ps, dt, eo, ew):
                x32t = x32p.tile([P, 512], F32, tag="x32t", name=f"xo{dt}_{eo}")
                nc.sync.dma_start(
                    x32t[:, :ew],
                    io["x32"].rearrange("(t p) e -> p t e", p=P)[:, dt, eo:eo + ew],
                )
                nc.vector.tensor_add(
                    mh_sb[:, dt, eo:eo + ew], ps[:, :ew], x32t[:, :ew]
                )

            proj_de(o_normT, w, bias_idx=_bias(BI_BO), evict=o_evict)
        p1s.close()  # free qT_sb / o_normT

        # ================= PHASE 2 =================
        qt_sb = ph2.tile([P, DT, C], BF16, name="qt_sb")
        xo_sb = ph2.tile([P, DT, C], BF16, name="xo_sb")

        with ExitStack() as s4:
            wpool = s4.enter_context(tc.tile_pool(name="wpool4", bufs=1))
            xtp = s4.enter_context(tc.tile_pool(name="xtp", bufs=1))
            xnT = xtp.tile([P, CT, D], BF16, tag="xnT", name="xnT")

            with ExitStack() as s4a:
                lnp = s4a.enter_context(tc.tile_pool(name="lnp", bufs=2))
                for dt in range(DT):
                    row = mh_sb[:, dt, :]
                    ssum = lnp.tile([P, 1], F32, tag="s1", name=f"su{dt}")
                    nc.vector.reduce_sum(ssum[:], row, axis=mybir.AxisListType.X)
                    mu = lnp.tile([P, 1], F32, tag="s2", name=f"mu{dt}")
                    nc.vector.tensor_scalar_mul(mu[:], ssum[:], 1.0 / C)
                    xm = lnp.tile([P, C], F32, tag="xm", name=f"xm{dt}")
                    nc.vector.tensor_scalar(
                        xm[:], row, mu[:], None, mybir.AluOpType.subtract
                    )
                    sq = lnp.tile([P, C], BF16, tag="sq", name=f"sq{dt}")
                    ssq = lnp.tile([P, 1], F32, tag="s3", name=f"sv{dt}")
                    nc.scalar.activation(
                        sq[:], xm[:], mybir.ActivationFunctionType.Square,
                        bias=zb[:], accum_out=ssq[:],
                    )
                    stdt = lnp.tile([P, 1], F32, tag="s4", name=f"sd{dt}")
                    nc.scalar.activation(
                        stdt[:], ssq[:], mybir.ActivationFunctionType.Sqrt,
                        scale=1.0 / C, bias=epsb[:],
                    )
                    rstd = lnp.tile([P, 1], F32, tag="s5", name=f"rs{dt}")
                    nc.vector.reciprocal(rstd[:], stdt[:])
                    xn_t = lnp.tile([P, C], BF16, tag="xn", name=f"xn{dt}")
                    nc.vector.tensor_scalar_mul(xn_t[:], xm[:], rstd[:])
                    # transpose this row-tile straight into xnT
                    for ct in range(CT):
                        tp = pt.tile([P, P], BF16, tag="tp", name=f"tn{dt}_{ct}")
                        nc.tensor.transpose(
                            tp[:, :], xn_t[:, ct * P:(ct + 1) * P], identity[:]
                        )
                        nc.any.tensor_copy(
                            xnT[:, ct, dt * P:(dt + 1) * P], tp[:, :]
                        )

            # ---- temporal k/v/q projections ----
            with ExitStack() as s4b:
                kvtp = s4b.enter_context(tc.tile_pool(name="kvtp", bufs=1))
                kt_sb = kvtp.tile([P, DT, C], BF16, tag="kt", name="kt_sb")
                vt_sb = kvtp.tile([P, DT, C], BF16, tag="vt", name="vt_sb")

                w = load_w(wpool, "wtkT")

                def kt_evict(ps, dt, eo, ew):
                    evict_copy(kt_sb[:, dt, eo:eo + ew], ps[:, :ew])

                proj_de(xnT, w, bias_idx=_bias(BI_BTK), evict=kt_evict)
                nc.sync.dma_start(
                    ag2k_in.rearrange("(t p e) -> p t e", p=P, e=C), kt_sb[:]
                )
                allgather(ag2k_in, ag2k_out, RG_FRAME)

                w = load_w(wpool, "wtvT")

                def vt_evict(ps, dt, eo, ew):
                    evict_copy(vt_sb[:, dt, eo:eo + ew], ps[:, :ew])

                proj_de(xnT, w, bias_idx=_bias(BI_BTV), evict=vt_evict)
                nc.sync.dma_start(
                    ag2v_in.rearrange("(t p e) -> p t e", p=P, e=C), vt_sb[:]
                )
                allgather(ag2v_in, ag2v_out, RG_FRAME)

                w = load_w(wpool, "wtqT")

                def qt_evict(ps, dt, eo, ew):
                    evict_copy(qt_sb[:, dt, eo:eo + ew], ps[:, :ew])

                proj_de(xnT, w, bias_idx=_bias(BI_BTQ), evict=qt_evict)

        # ---- temporal attention (seq len 2 -> sigmoid weights) ----
        with ExitStack() as s5:
            kvp = s5.enter_context(tc.tile_pool(name="kvp", bufs=3))
            ap = s5.enter_context(tc.tile_pool(name="ap", bufs=3))
            for dt in range(DT):
                k0 = kvp.tile([P, C], BF16, tag="k0", name=f"k0_{dt}")
                k1 = kvp.tile([P, C], BF16, tag="k1", name=f"k1_{dt}")
                v0 = kvp.tile([P, C], BF16, tag="v0", name=f"v0_{dt}")
                v1 = kvp.tile([P, C], BF16, tag="v1", name=f"v1_{dt}")
                for t, src_t in zip((k0, k1, v0, v1),
                                    (ag2k_out[0], ag2k_out[1],
                                     ag2v_out[0], ag2v_out[1])):
                    nc.sync.dma_start(
                        t[:],
                        src_t.rearrange("(t p e) -> p t e", p=P, e=C)[:, dt],
                    )
                qrow = qt_sb[:, dt, :]
                prod = ap.tile([P, C], F32, tag="prod", name=f"pr{dt}")
                s0 = ap.tile([P, HEADS], F32, tag="s0", name=f"s0_{dt}")
                s1v = ap.tile([P, HEADS], F32, tag="s1v", name=f"s1_{dt}")
                nc.vector.tensor_mul(prod[:], qrow, k0[:])
                nc.vector.reduce_sum(
                    s0[:], prod.rearrange("p (h w) -> p h w", w=DH),
                    axis=mybir.AxisListType.X,
                )
                nc.vector.tensor_mul(prod[:], qrow, k1[:])
                nc.vector.reduce_sum(
                    s1v[:], prod.rearrange("p (h w) -> p h w", w=DH),
                    axis=mybir.AxisListType.X,
                )
                sd = ap.tile([P, HEADS], F32, tag="sd", name=f"sd{dt}")
                nc.vector.tensor_sub(sd[:], s1v[:], s0[:])
                w1 = ap.tile([P, HEADS], F32, tag="w1", name=f"w1_{dt}")
                nc.scalar.activation(
                    w1[:], sd[:], mybir.ActivationFunctionType.Sigmoid,
                    bias=zb[:], scale=SCALE,
                )
                dv = ap.tile([P, C], F32, tag="dv", name=f"dv{dt}")
                nc.vector.tensor_sub(dv[:], v1[:], v0[:])
                # xo = v0 + w1 * (v1 - v0), per 64-wide head segment
                for hh in range(HEADS):
                    seg = slice(hh * DH, (hh + 1) * DH)
                    nc.vector.tensor_scalar_mul(
                        prod[:, seg], dv[:, seg], w1[:, hh:hh + 1]
                    )
                    nc.vector.tensor_add(xo_sb[:, dt, seg], prod[:, seg], v0[:, seg])

        # ---- final projection + residual ----
        with ExitStack() as s6:
            wpool = s6.enter_context(tc.tile_pool(name="wpool6", bufs=1))
            xtp6 = s6.enter_context(tc.tile_pool(name="xtp6", bufs=1))
            fin = s6.enter_context(tc.tile_pool(name="fin", bufs=3))

            xoT = xtp6.tile([P, CT, D], BF16, name="xoT")
            transpose_into(xoT, xo_sb, DT, CT)
            w = load_w(wpool, "wtoT")

            def fin_evict(ps, dt, eo, ew):
                x32t = fin.tile([P, 512], F32, tag="x32t", name=f"xx{dt}_{eo}")
                nc.sync.dma_start(
                    x32t[:, :ew],
                    io["x32"].rearrange("(t p) e -> p t e", p=P)[:, dt, eo:eo + ew],
                )
                ot = fin.tile([P, 512], F32, tag="ot", name=f"ot{dt}_{eo}")
                nc.vector.tensor_add(ot[:, :ew], ps[:, :ew], mh_sb[:, dt, eo:eo + ew])
                nc.vector.tensor_sub(ot[:, :ew], ot[:, :ew], x32t[:, :ew])
                nc.sync.dma_start(
                    out_dram.rearrange("(t p) e -> p t e", p=P)[:, dt, eo:eo + ew],
                    ot[:, :ew],
                )

            proj_de(xoT, w, bias_idx=_bias(BI_BTO), evict=fin_evict)


_NC_CACHE = None


def _get_program():
    global _NC_CACHE
    if _NC_CACHE is None:
        _NC_CACHE = _build_program()
    return _NC_CACHE


def _prep_in_maps(h, Wq, Wk, Wv, Wo, bo, Dq, Uq, Dk, Uk, Dv, Uv, Do, Uo,
                  gamma, beta, Wi, bi, Wtq, btq, Wtk, btk, Wtv, btv, Wto, bto):
    def tb(x):
        return np.ascontiguousarray(np.asarray(x, np.float32).T).astype(NPBF)

    gamma = np.asarray(gamma, np.float32)
    beta = np.asarray(beta, np.float32)
    Wi = np.asarray(Wi, np.float32)
    WiP = Wi * gamma[None, :]
    biP = np.asarray(bi, np.float32) + Wi @ beta

    Wtq = np.asarray(Wtq, np.float32); Wtk = np.asarray(Wtk, np.float32)
    Wtv = np.asarray(Wtv, np.float32)
    shared = {
        "wtqT": tb(Wtq @ WiP), "wtkT": tb(Wtk @ WiP), "wtvT": tb(Wtv @ WiP),
        "wtoT": tb(Wto),
        "biases": np.stack([
            np.asarray(bo, np.float32), biP,
            np.asarray(btq, np.float32) + Wtq @ biP,
            np.asarray(btk, np.float32) + Wtk @ biP,
            np.asarray(btv, np.float32) + Wtv @ biP,
            np.asarray(bto, np.float32),
        ])[None].astype(NPBF),
    }
    # merge LoRA into the shared weights per frame: W_eff = W + U[f] @ D[f]
    def eff(Wm, Um, Dm, f):
        return tb(np.asarray(Wm, np.float32)
                  + np.asarray(Um[f], np.float32) @ np.asarray(Dm[f], np.float32))

    lora = []
    for f in range(FRAMES):
        lora.append({
            "wqT": eff(Wq, Uq, Dq, f), "wkT": eff(Wk, Uk, Dk, f),
            "wvT": eff(Wv, Uv, Dv, f), "woT": eff(Wo, Uo, Do, f),
        })

    h = np.asarray(h, np.float32)
    in_maps = []
    for core in range(8):
        f, b, dh = core // 4, (core // 2) % 2, core % 2
        x32 = np.ascontiguousarray(h[b * FRAMES + f, dh * D:(dh + 1) * D, :])
        m = dict(shared)
        m.update(lora[f])
        m["x32"] = x32
        m["xT"] = np.ascontiguousarray(x32.T).astype(NPBF)
        in_maps.append(m)
    return in_maps


def kernel(h, **kw):
    nc = _get_program()
    in_maps = _prep_in_maps(h, **kw)
    res = run_bass_kernel_spmd(nc, in_maps, list(range(8))).results
    out = np.empty((B * FRAMES, D_FULL, C), np.float32)
    for core in range(8):
        f, b, dh = core // 4, (core // 2) % 2, core % 2
        out[b * FRAMES + f, dh * D:(dh + 1) * D, :] = res[core]["out"]
    return out


# revision 36
# speedup vs baseline: 132.2316x; 1.3803x over previous
"""AttentionSharingUnit kernel for 8 Trainium2 cores (Bass/Tile).

Sharding: core = f*4 + b*2 + dh  (frame, batch, d-half). Each core owns 1024
rows of one (frame, batch) and keeps them for the whole kernel.

Host-side exact folds: LoRA merged into the projection weights per frame
(W_eff = W + U[f] @ D[f]); LayerNorm gamma/beta and the Wi projection folded
into the temporal q/k/v weights (Wt*_eff = Wt* @ (Wi*gamma), biases folded
likewise); biases enter as K=1 matmul rows.

Phase 1 (spatial attn): q/k/v projections in bf16 on TensorE, two 2-rank
AllGathers (kT then v) across d-half pairs, each issued right after its
producer so later projections overlap the collective; per-head attention
with "lazy softmax" (no max subtraction -- max |sim*scale| ~ 9; the
normalizer falls out of the PV matmul against a ones column appended to V);
O-projection and residual. The residual uses the same bf16 x for the add
here and the subtract at the end, so the input cancels exactly and no fp32
copy of h is needed on device.

Phase 2 (temporal attn over 2 frames): LayerNorm, temporal q/k/v straight
from the normalized activations, two 2-rank AllGathers (kt, vt) across
frame pairs overlapped with the qt projection, the 2-way softmax collapsed
to a sigmoid on ScalarE, Wto projection, final residual.

All matmul inputs bf16 with fp32 PSUM accumulation; residual path fp32.
"""

import os
import sys
from contextlib import ExitStack

import numpy as np

sys.path.insert(0, "/opt/trn_rl_repo")

import ml_dtypes

import concourse.bass as bass
import concourse.tile as tile
from concourse import bacc, mybir
from concourse.bass_utils import run_bass_kernel_spmd
from concourse.masks import make_identity

BF16 = mybir.dt.bfloat16
F32 = mybir.dt.float32
NPBF = ml_dtypes.bfloat16

FRAMES = 2
HEADS = 20
C = 1280
RANK = 256
B = 2
D_FULL = 2048
D = 1024          # rows per core
P = 128
CT = C // P       # 10 c-chunks
DT = D // P       # 8 d-tiles per core
RT = RANK // P    # 2 r-chunks
DH = 64           # head dim
NJ = 16           # j-chunks of 128 over full 2048 keys
EPS = 1e-6
SCALE = DH ** -0.5

EB = [(0, 512), (512, 512), (1024, 256)]   # e-blocks covering 1280
BI_BO, BI_BIP, BI_BTQ, BI_BTK, BI_BTV, BI_BTO = range(6)

RG_DHALF = [[0, 1], [2, 3], [4, 5], [6, 7]]   # phase-1 allgather groups
RG_FRAME = [[0, 4], [1, 5], [2, 6], [3, 7]]   # phase-2 allgather groups

HALF_ELEMS = CT * P * D  # 1310720 elements per packed AG slot

# When True, collectives are replaced by local DRAM->DRAM copies with the
# same buffer sizes (for single-core TimelineSim cost-model introspection).
EMULATE_COLLECTIVES = False
# When True, omit the K=1 bias matmuls (exact when all biases are zero;
# decided on the host per input values).
SKIP_BIAS = False


def _build_program(n_iters=1):
    nc = bacc.Bacc("TRN2", target_bir_lowering=False, debug=False, num_devices=8)

    def din(name, shape, dt=BF16):
        return nc.dram_tensor(name, list(shape), dt, kind="ExternalInput").ap()

    io = {}
    io["xT"] = din("xT", (C, D))
    for w in ("wqT", "wkT", "wvT", "woT", "wtqT", "wtkT", "wtvT", "wtoT"):
        io[w] = din(w, (C, C))
    io["biases"] = din("biases", (1, 6, C))
    out_dram = nc.dram_tensor("out", [D, C], F32, kind="ExternalOutput").ap()

    with tile.TileContext(nc) as tc:
        for _ in range(n_iters):
            _emit(tc, nc, io, out_dram)
    nc.compile()
    return nc


def _bias(i):
    return None if SKIP_BIAS else i


def _emit(tc, nc, io, out_dram):
    with ExitStack() as top:
        const_pool = top.enter_context(tc.tile_pool(name="const", bufs=1))
        identity = const_pool.tile([P, P], BF16, name="identity")
        make_identity(nc, identity[:])
        ones_row = const_pool.tile([1, 512], BF16, name="ones_row")
        nc.vector.memset(ones_row[:], 1.0)
        zb = const_pool.tile([P, 1], F32, name="zb")
        nc.vector.memset(zb[:], 0.0)
        epsb = const_pool.tile([P, 1], F32, name="epsb")
        nc.vector.memset(epsb[:], EPS)
        bias_sb = const_pool.tile([1, 6, C], BF16, name="bias_sb")
        nc.sync.dma_start(bias_sb[:], io["biases"][:])

        # PSUM pools (8 banks total)
        pp = top.enter_context(tc.tile_pool(name="pp", bufs=3, space="PSUM"))
        po = top.enter_context(tc.tile_pool(name="po", bufs=3, space="PSUM"))
        pt = top.enter_context(tc.tile_pool(name="pt", bufs=2, space="PSUM"))

        dram = top.enter_context(tc.tile_pool(name="dram", bufs=1, space="DRAM"))
        ag1k_in = dram.tile([HALF_ELEMS], BF16, name="ag1k_in")
        ag1k_out = dram.tile([2, HALF_ELEMS], BF16, name="ag1k_out")
        ag1v_in = dram.tile([HALF_ELEMS], BF16, name="ag1v_in")
        ag1v_out = dram.tile([2, HALF_ELEMS], BF16, name="ag1v_out")
        ag2k_in = dram.tile([HALF_ELEMS], BF16, name="ag2k_in")
        ag2k_out = dram.tile([2, HALF_ELEMS], BF16, name="ag2k_out")
        ag2v_in = dram.tile([HALF_ELEMS], BF16, name="ag2v_in")
        ag2v_out = dram.tile([2, HALF_ELEMS], BF16, name="ag2v_out")

        def allgather(in_t, out_t, groups):
            if EMULATE_COLLECTIVES:
                nc.sync.dma_start(out_t[0], in_t[:])
                nc.sync.dma_start(out_t[1], in_t[:])
            else:
                nc.gpsimd.collective_compute(
                    "AllGather",
                    mybir.AluOpType.bypass,
                    replica_groups=groups,
                    ins=[in_t[:].opt()],
                    outs=[out_t[:].opt()],
                )

        # ---------- generic projection emitters ----------
        _evict_rr = [0]

        def evict_copy(dst, src_ps):
            _evict_rr[0] ^= 1
            eng = nc.scalar if _evict_rr[0] else nc.vector
            if eng is nc.scalar:
                nc.scalar.copy(dst, src_ps)
            else:
                nc.vector.tensor_copy(dst, src_ps)

        def proj_ed(out_sb, nt, x_sb, w_sb, lora=None, bias_idx=None):
            # out[e|r, d] = W.T @ xT : out_sb [P, nt, D]; x_sb [P, CT, D];
            # w_sb [P, CT, nt*P]
            nk = CT + (RT if lora else 0) + (1 if bias_idx is not None else 0)
            for et in range(nt):
                for db in range(2):
                    ps = pp.tile([P, 512], F32, tag="big", name=f"ps{et}_{db}")
                    k = 0
                    for ct in range(CT):
                        k += 1
                        nc.tensor.matmul(
                            ps[:, :],
                            w_sb[:, ct, et * P:(et + 1) * P],
                            x_sb[:, ct, db * 512:(db + 1) * 512],
                            start=(k == 1), stop=(k == nk),
                        )
                    if lora is not None:
                        tT_sb, u_sb = lora
                        for rt in range(RT):
                            k += 1
                            nc.tensor.matmul(
                                ps[:, :],
                                u_sb[:, rt, et * P:(et + 1) * P],
                                tT_sb[:, rt, db * 512:(db + 1) * 512],
                                start=(k == 1), stop=(k == nk),
                            )
                    if bias_idx is not None:
                        k += 1
                        nc.tensor.matmul(
                            ps[:, :],
                            bias_sb[0:1, bias_idx, et * P:(et + 1) * P],
                            ones_row[0:1, 0:512],
                            start=(k == 1), stop=(k == nk),
                        )
                    evict_copy(out_sb[:, et, db * 512:(db + 1) * 512], ps[:, :])

        def proj_de(x_sb, w_sb, lora=None, bias_idx=None, evict=None):
            # out[d, e] = xT.T @ W : x_sb [P, CT, D]; w_sb [P, CT, C]
            nk = CT + (RT if lora else 0) + (1 if bias_idx is not None else 0)
            for dt in range(DT):
                for (eo, ew) in EB:
                    ps = pp.tile([P, 512], F32, tag="big", name=f"pd{dt}_{eo}")
                    k = 0
                    for ct in range(CT):
                        k += 1
                        nc.tensor.matmul(
                            ps[:, :ew],
                            x_sb[:, ct, dt * P:(dt + 1) * P],
                            w_sb[:, ct, eo:eo + ew],
                            start=(k == 1), stop=(k == nk),
                        )
                    if lora is not None:
                        tT_sb, u_sb = lora
                        for rt in range(RT):
                            k += 1
                            nc.tensor.matmul(
                                ps[:, :ew],
                                tT_sb[:, rt, dt * P:(dt + 1) * P],
                                u_sb[:, rt, eo:eo + ew],
                                start=(k == 1), stop=(k == nk),
                            )
                    if bias_idx is not None:
                        k += 1
                        nc.tensor.matmul(
                            ps[:, :ew],
                            ones_row[0:1, 0:P],
                            bias_sb[0:1, bias_idx, eo:eo + ew],
                            start=(k == 1), stop=(k == nk),
                        )
                    evict(ps, dt, eo, ew)

        def transpose_into(dst_sb, src_sb, nt_src, nt_dst):
            # src [P, nt_src, nt_dst*P] -> dst [P, nt_dst, nt_src*P]
            for st in range(nt_src):
                for ot in range(nt_dst):
                    tp = pt.tile([P, P], BF16, tag="tp", name=f"tp{st}_{ot}")
                    nc.tensor.transpose(
                        tp[:, :], src_sb[:, st, ot * P:(ot + 1) * P], identity[:]
                    )
                    nc.any.tensor_copy(dst_sb[:, ot, st * P:(st + 1) * P], tp[:, :])

        def load_w(pool, name, tag="wfull"):
            t = pool.tile([P, CT, C], BF16, tag=tag, name=f"w_{name}")
            nc.sync.dma_start(t[:], io[name].rearrange("(t p) e -> p t e", p=P))
            return t

        # ================= PHASE 1 =================
        p1s = ExitStack()
        ph1 = p1s.enter_context(tc.tile_pool(name="ph1", bufs=1))
        qT_sb = ph1.tile([P, CT, D], BF16, name="qT_sb")
        o_normT = ph1.tile([P, CT, D], BF16, name="o_normT")
        xT_sb = ph1.tile([P, CT, D], BF16, name="xT_sb")

        with ExitStack() as s1:
            wpool = s1.enter_context(tc.tile_pool(name="wpool1", bufs=2))
            kvpool = s1.enter_context(tc.tile_pool(name="kvpool1", bufs=1))

            nc.sync.dma_start(xT_sb[:], io["xT"].rearrange("(t p) d -> p t d", p=P))

            # ---- k projection (own half) ----
            w = load_w(wpool, "wkT")
            kT_own = kvpool.tile([P, CT, D], BF16, tag="kvown", name="kT_own")
            proj_ed(kT_own, CT, xT_sb, w)
            nc.sync.dma_start(
                ag1k_in.rearrange("(t p d) -> p t d", p=P, d=D), kT_own[:]
            )
            allgather(ag1k_in, ag1k_out, RG_DHALF)

            # ---- v projection (own half) ----
            w = load_w(wpool, "wvT")
            v_own = kvpool.tile([P, DT, C], BF16, tag="kvown", name="v_own")

            def v_evict(ps, dt, eo, ew):
                evict_copy(v_own[:, dt, eo:eo + ew], ps[:, :ew])

            proj_de(xT_sb, w, evict=v_evict)
            nc.sync.dma_start(
                ag1v_in.rearrange("(t p e) -> p t e", p=P, e=C), v_own[:]
            )
            allgather(ag1v_in, ag1v_out, RG_DHALF)

            # ---- q projection (own rows; overlaps the collective) ----
            w = load_w(wpool, "wqT")
            proj_ed(qT_sb, CT, xT_sb, w)

        # ---- attention: stream per-head-pair k/v from the AG dram buffer ----
        with ExitStack() as s2:
            khp = s2.enter_context(tc.tile_pool(name="khp", bufs=2))
            vhp = s2.enter_context(tc.tile_pool(name="vhp", bufs=3))
            epool = s2.enter_context(tc.tile_pool(name="epool", bufs=3))
            spool = s2.enter_context(tc.tile_pool(name="spool", bufs=4))
            opp = s2.enter_context(tc.tile_pool(name="opp", bufs=2))
            for hc in range(CT):  # head-pair = one 128-channel chunk
                kh2 = khp.tile([P, D_FULL], BF16, tag="kh", name=f"kh{hc}")
                for g in range(2):
                    nc.sync.dma_start(
                        kh2[:, g * D:(g + 1) * D],
                        ag1k_out[g].rearrange("(t p d) -> p t d", p=P, d=D)[:, hc],
                    )
                o_pair = opp.tile([P, DT, P], BF16, tag="opair", name=f"op{hc}")
                for h01 in range(2):
                    hh = 2 * hc + h01
                    poff = h01 * DH
                    vh = vhp.tile([P, NJ, 65], BF16, tag="vh", name=f"vh{hh}")
                    for g in range(2):
                        nc.sync.dma_start(
                            vh[:, g * DT:(g + 1) * DT, 0:DH],
                            ag1v_out[g].rearrange("(t p e) -> p t e", p=P, e=C)[
                                :, :, hh * DH:(hh + 1) * DH
                            ],
                        )
                    nc.vector.memset(vh[:, :, DH:65], 1.0)
                    for ib in range(2):
                        exp_t = epool.tile(
                            [P, NJ, 512], BF16, tag="exp", name=f"e{hh}_{ib}"
                        )
                        for jc in range(NJ):
                            sim = pp.tile(
                                [P, 512], F32, tag="big", name=f"s{hh}{ib}{jc}"
                            )
                            nc.tensor.matmul(
                                sim[:, :],
                                kh2[poff:poff + DH, jc * P:(jc + 1) * P],
                                qT_sb[poff:poff + DH, hc,
                                      ib * 512:(ib + 1) * 512],
                                start=True, stop=True,
                            )
                            nc.scalar.activation(
                                exp_t[:, jc, :], sim[:, :],
                                mybir.ActivationFunctionType.Exp,
                                bias=zb[:], scale=SCALE,
                            )
                        # PV with exp stationary (N=65 streams; LDWEIGHTS are
                        # hidden by the engine queue, so this is stream-optimal)
                        for it in range(4):
                            dt = ib * 4 + it
                            ops = po.tile(
                                [P, 65], F32, tag="opsum", name=f"o{hh}_{dt}"
                            )
                            for jc in range(NJ):
                                nc.tensor.matmul(
                                    ops[:, :],
                                    exp_t[:, jc, it * P:(it + 1) * P],
                                    vh[:, jc, :],
                                    start=(jc == 0), stop=(jc == NJ - 1),
                                )
                            rec = spool.tile(
                                [P, 1], F32, tag="rec", name=f"r{hh}_{dt}"
                            )
                            nc.vector.reciprocal(rec[:], ops[:, DH:65])
                            nc.vector.tensor_scalar_mul(
                                o_pair[:, dt, poff:poff + DH], ops[:, 0:DH], rec[:]
                            )
                # transpose the pair's [d, 128c] block into o_normT [c, d]
                for dt in range(DT):
                    tp = pt.tile([P, P], BF16, tag="tp", name=f"t{hc}_{dt}")
                    nc.tensor.transpose(tp[:, :], o_pair[:, dt, :], identity[:])
                    nc.any.tensor_copy(
                        o_normT[:, hc, dt * P:(dt + 1) * P], tp[:, :]
                    )

        # ---- O projection + residual -> mh ----
        ph2 = top.enter_context(tc.tile_pool(name="ph2", bufs=1, side="right"))
        mh_sb = ph2.tile([P, DT, C], F32, name="mh_sb")

        xb_sb = ph2.tile([P, DT, C], BF16, name="xb_sb")
        with ExitStack() as s3:
            wpool = s3.enter_context(tc.tile_pool(name="wpool3", bufs=1))

            transpose_into(xb_sb, xT_sb, CT, DT)
            w = load_w(wpool, "woT")

            def o_evict(ps, dt, eo, ew):
                nc.vector.tensor_add(
                    mh_sb[:, dt, eo:eo + ew], ps[:, :ew],
                    xb_sb[:, dt, eo:eo + ew],
                )

            proj_de(o_normT, w, bias_idx=_bias(BI_BO), evict=o_evict)
        p1s.close()  # free qT_sb / o_normT / xT_sb

        # ================= PHASE 2 =================
        qt_sb = ph2.tile([P, DT, C], BF16, name="qt_sb")
        xo_sb = ph2.tile([P, DT, C], BF16, name="xo_sb")

        with ExitStack() as s4:
            wpool = s4.enter_context(tc.tile_pool(name="wpool4", bufs=1))
            xtp = s4.enter_context(tc.tile_pool(name="xtp", bufs=1))
            xnT = xtp.tile([P, CT, D], BF16, tag="xnT", name="xnT")

            with ExitStack() as s4a:
                lnp = s4a.enter_context(tc.tile_pool(name="lnp", bufs=2))
                for dt in range(DT):
                    row = mh_sb[:, dt, :]
                    ssum = lnp.tile([P, 1], F32, tag="s1", name=f"su{dt}")
                    nc.vector.reduce_sum(ssum[:], row, axis=mybir.AxisListType.X)
                    mu = lnp.tile([P, 1], F32, tag="s2", name=f"mu{dt}")
                    nc.vector.tensor_scalar_mul(mu[:], ssum[:], 1.0 / C)
                    xm = lnp.tile([P, C], F32, tag="xm", name=f"xm{dt}")
                    nc.vector.tensor_scalar(
                        xm[:], row, mu[:], None, mybir.AluOpType.subtract
                    )
                    sq = lnp.tile([P, C], BF16, tag="sq", name=f"sq{dt}")
                    ssq = lnp.tile([P, 1], F32, tag="s3", name=f"sv{dt}")
                    nc.scalar.activation(
                        sq[:], xm[:], mybir.ActivationFunctionType.Square,
                        bias=zb[:], accum_out=ssq[:],
                    )
                    stdt = lnp.tile([P, 1], F32, tag="s4", name=f"sd{dt}")
                    nc.scalar.activation(
                        stdt[:], ssq[:], mybir.ActivationFunctionType.Sqrt,
                        scale=1.0 / C, bias=epsb[:],
                    )
                    rstd = lnp.tile([P, 1], F32, tag="s5", name=f"rs{dt}")
                    nc.vector.reciprocal(rstd[:], stdt[:])
                    xn_t = lnp.tile([P, C], BF16, tag="xn", name=f"xn{dt}")
                    nc.vector.tensor_scalar_mul(xn_t[:], xm[:], rstd[:])
                    # transpose this row-tile straight into xnT
                    for ct in range(CT):
                        tp = pt.tile([P, P], BF16, tag="tp", name=f"tn{dt}_{ct}")
                        nc.tensor.transpose(
                            tp[:, :], xn_t[:, ct * P:(ct + 1) * P], identity[:]
                        )
                        nc.any.tensor_copy(
                            xnT[:, ct, dt * P:(dt + 1) * P], tp[:, :]
                        )

            # ---- temporal k/v/q projections ----
            with ExitStack() as s4b:
                kvtp = s4b.enter_context(tc.tile_pool(name="kvtp", bufs=1))
                kt_sb = kvtp.tile([P, DT, C], BF16, tag="kt", name="kt_sb")
                vt_sb = kvtp.tile([P, DT, C], BF16, tag="vt", name="vt_sb")

                w = load_w(wpool, "wtkT")

                def kt_evict(ps, dt, eo, ew):
                    evict_copy(kt_sb[:, dt, eo:eo + ew], ps[:, :ew])

                proj_de(xnT, w, bias_idx=_bias(BI_BTK), evict=kt_evict)
                nc.sync.dma_start(
                    ag2k_in.rearrange("(t p e) -> p t e", p=P, e=C), kt_sb[:]
                )
                allgather(ag2k_in, ag2k_out, RG_FRAME)

                w = load_w(wpool, "wtvT")

                def vt_evict(ps, dt, eo, ew):
                    evict_copy(vt_sb[:, dt, eo:eo + ew], ps[:, :ew])

                proj_de(xnT, w, bias_idx=_bias(BI_BTV), evict=vt_evict)
                nc.sync.dma_start(
                    ag2v_in.rearrange("(t p e) -> p t e", p=P, e=C), vt_sb[:]
                )
                allgather(ag2v_in, ag2v_out, RG_FRAME)

                w = load_w(wpool, "wtqT")

                def qt_evict(ps, dt, eo, ew):
                    evict_copy(qt_sb[:, dt, eo:eo + ew], ps[:, :ew])

                proj_de(xnT, w, bias_idx=_bias(BI_BTQ), evict=qt_evict)

        # ---- temporal attention (seq len 2 -> sigmoid weights) ----
        with ExitStack() as s5:
            kvp = s5.enter_context(tc.tile_pool(name="kvp", bufs=3))
            ap = s5.enter_context(tc.tile_pool(name="ap", bufs=3))
            for dt in range(DT):
                k0 = kvp.tile([P, C], BF16, tag="k0", name=f"k0_{dt}")
                k1 = kvp.tile([P, C], BF16, tag="k1", name=f"k1_{dt}")
                v0 = kvp.tile([P, C], BF16, tag="v0", name=f"v0_{dt}")
                v1 = kvp.tile([P, C], BF16, tag="v1", name=f"v1_{dt}")
                for t, src_t in zip((k0, k1, v0, v1),
                                    (ag2k_out[0], ag2k_out[1],
                                     ag2v_out[0], ag2v_out[1])):
                    nc.sync.dma_start(
                        t[:],
                        src_t.rearrange("(t p e) -> p t e", p=P, e=C)[:, dt],
                    )
                qrow = qt_sb[:, dt, :]
                prod = ap.tile([P, C], F32, tag="prod", name=f"pr{dt}")
                s0 = ap.tile([P, HEADS], F32, tag="s0", name=f"s0_{dt}")
                s1v = ap.tile([P, HEADS], F32, tag="s1v", name=f"s1_{dt}")
                nc.vector.tensor_mul(prod[:], qrow, k0[:])
                nc.vector.reduce_sum(
                    s0[:], prod.rearrange("p (h w) -> p h w", w=DH),
                    axis=mybir.AxisListType.X,
                )
                nc.vector.tensor_mul(prod[:], qrow, k1[:])
                nc.vector.reduce_sum(
                    s1v[:], prod.rearrange("p (h w) -> p h w", w=DH),
                    axis=mybir.AxisListType.X,
                )
                sd = ap.tile([P, HEADS], F32, tag="sd", name=f"sd{dt}")
                nc.vector.tensor_sub(sd[:], s1v[:], s0[:])
                w1 = ap.tile([P, HEADS], F32, tag="w1", name=f"w1_{dt}")
                nc.scalar.activation(
                    w1[:], sd[:], mybir.ActivationFunctionType.Sigmoid,
                    bias=zb[:], scale=SCALE,
                )
                dv = ap.tile([P, C], F32, tag="dv", name=f"dv{dt}")
                nc.vector.tensor_sub(dv[:], v1[:], v0[:])
                # xo = v0 + w1 * (v1 - v0), per 64-wide head segment
                for hh in range(HEADS):
                    seg = slice(hh * DH, (hh + 1) * DH)
                    nc.vector.tensor_scalar_mul(
                        prod[:, seg], dv[:, seg], w1[:, hh:hh + 1]
                    )
                    nc.vector.tensor_add(xo_sb[:, dt, seg], prod[:, seg], v0[:, seg])

        # ---- final projection + residual ----
        with ExitStack() as s6:
            wpool = s6.enter_context(tc.tile_pool(name="wpool6", bufs=1))
            xtp6 = s6.enter_context(tc.tile_pool(name="xtp6", bufs=1))
            fin = s6.enter_context(tc.tile_pool(name="fin", bufs=3))

            xoT = xtp6.tile([P, CT, D], BF16, name="xoT")
            transpose_into(xoT, xo_sb, DT, CT)
            w = load_w(wpool, "wtoT")

            def fin_evict(ps, dt, eo, ew):
                ot = fin.tile([P, 512], F32, tag="ot", name=f"ot{dt}_{eo}")
                nc.vector.tensor_add(ot[:, :ew], ps[:, :ew], mh_sb[:, dt, eo:eo + ew])
                nc.vector.tensor_sub(
                    ot[:, :ew], ot[:, :ew], xb_sb[:, dt, eo:eo + ew]
                )
                nc.sync.dma_start(
                    out_dram.rearrange("(t p) e -> p t e", p=P)[:, dt, eo:eo + ew],
                    ot[:, :ew],
                )

            proj_de(xoT, w, bias_idx=_bias(BI_BTO), evict=fin_evict)


_NC_CACHE = None


def _get_program():
    global _NC_CACHE
    if _NC_CACHE is None:
        _NC_CACHE = _build_program()
    return _NC_CACHE


def _prep_in_maps(h, Wq, Wk, Wv, Wo, bo, Dq, Uq, Dk, Uk, Dv, Uv, Do, Uo,
                  gamma, beta, Wi, bi, Wtq, btq, Wtk, btk, Wtv, btv, Wto, bto):
    def tb(x):
        return np.ascontiguousarray(np.asarray(x, np.float32).T).astype(NPBF)

    gamma = np.asarray(gamma, np.float32)
    beta = np.asarray(beta, np.float32)
    Wi = np.asarray(Wi, np.float32)
    WiP = Wi * gamma[None, :]
    biP = np.asarray(bi, np.float32) + Wi @ beta

    Wtq = np.asarray(Wtq, np.float32); Wtk = np.asarray(Wtk, np.float32)
    Wtv = np.asarray(Wtv, np.float32)
    shared = {
        "wtqT": tb(Wtq @ WiP), "wtkT": tb(Wtk @ WiP), "wtvT": tb(Wtv @ WiP),
        "wtoT": tb(Wto),
        "biases": np.stack([
            np.asarray(bo, np.float32), biP,
            np.asarray(btq, np.float32) + Wtq @ biP,
            np.asarray(btk, np.float32) + Wtk @ biP,
            np.asarray(btv, np.float32) + Wtv @ biP,
            np.asarray(bto, np.float32),
        ])[None].astype(NPBF),
    }
    # merge LoRA into the shared weights per frame: W_eff = W + U[f] @ D[f]
    def eff(Wm, Um, Dm, f):
        return tb(np.asarray(Wm, np.float32)
                  + np.asarray(Um[f], np.float32) @ np.asarray(Dm[f], np.float32))

    lora = []
    for f in range(FRAMES):
        lora.append({
            "wqT": eff(Wq, Uq, Dq, f), "wkT": eff(Wk, Uk, Dk, f),
            "wvT": eff(Wv, Uv, Dv, f), "woT": eff(Wo, Uo, Do, f),
        })

    h = np.asarray(h, np.float32)
    in_maps = []
    for core in range(8):
        f, b, dh = core // 4, (core // 2) % 2, core % 2
        x32 = np.ascontiguousarray(h[b * FRAMES + f, dh * D:(dh + 1) * D, :])
        m = dict(shared)
        m.update(lora[f])
        m["xT"] = np.ascontiguousarray(x32.T).astype(NPBF)
        in_maps.append(m)
    return in_maps


def kernel(h, **kw):
    nc = _get_program()
    in_maps = _prep_in_maps(h, **kw)
    res = run_bass_kernel_spmd(nc, in_maps, list(range(8))).results
    out = np.empty((B * FRAMES, D_FULL, C), np.float32)
    for core in range(8):
        f, b, dh = core // 4, (core // 2) % 2, core % 2
        out[b * FRAMES + f, dh * D:(dh + 1) * D, :] = res[core]["out"]
    return out
